# revision 2
# baseline (speedup 1.0000x reference)
"""DTransformer forward on 8 trn2 NeuronCores — v2 (fp16 + stage batching).

Sharding: core c = batch c//4, head pair c%4 (heads 2p, 2p+1) of all three
blocks, as v1. Changes vs v1:
- All matmul operands fp16 (was fp32r for projections/Wo/ctx) — full PE
  rate, half the DMA and collective bytes.
- Scores Exp reads batched [128,1024] 2-bank PSUM tiles (24 ACT calls per
  unit instead of 40); score fp16 copies batched the same way.
- Full-unit stage batching (Exp-all-tiles, Sqrt-all, Exp-all, Exp-all)
  with exact-size chain tiles -> 2 ACT table switches per unit (was 4+).
- Per-unit AllGathers (fp16, [32,2048] x6) so each collective overlaps the
  next unit's compute; collectives sit on the gpsimd queue right after
  each unit's stores.
- Phase-2 hq/ha rebuilds are emitted BETWEEN phase-1/3 units so their
  vector work fills the ACT-heavy units' vector slack.
- s2 = m0*sc on gpsimd (otherwise idle) for the 4 b1/b2 units.
- V bias folded into the csb copy (tensor_scalar add with a [32,1] column)
  instead of a per-position add on V.
- DMA transposes issued from the scalar queue (2nd HWDGE queue) to unload
  the sync queue.
"""
import sys

if "/opt/trn_rl_repo" not in sys.path:
    sys.path.insert(0, "/opt/trn_rl_repo")

import numpy as np

import concourse.bacc as bacc
import concourse.mybir as mybir
import concourse.tile as tile
from concourse import bass_utils

dt = mybir.dt
AF = mybir.ActivationFunctionType
ALU = mybir.AluOpType
AX = mybir.AxisListType

S, D, H, DK, B = 2048, 256, 8, 32, 2
NT = S // 128
ISQ = float(1.0 / np.sqrt(DK))
NEG = -30.0

_BUILT = {}


def _softplus(x):
    return np.logaddexp(0.0, x)


def build_in_maps(inp):
    f32, f16 = np.float32, np.float16
    q_emb = np.asarray(inp["q_emb"], f32)
    qa_emb = np.asarray(inp["qa_emb"], f32)

    pos16 = np.maximum(
        np.arange(128)[:, None] + 2048 - np.arange(4096)[None, :], 0
    ).astype(f16)
    spos16 = np.sqrt(pos16.astype(np.float64)).astype(f16)
    i_l = np.arange(128)[:, None]
    j_l = np.arange(128)[None, :]
    tri_pk = (j_l > i_l).astype(np.uint8)
    tri_st = (j_l >= i_l).astype(np.uint8)

    know = np.asarray(inp["know_params"], f32)[0, 0]
    q3 = know @ np.asarray(inp["b3_Wq"], f32) + np.asarray(inp["b3_bq"], f32)
    gam = {k: -_softplus(np.asarray(inp[k + "_gam"], f32)[:, 0, 0])
           for k in ("b1", "b2", "b3")}
    Wq = [np.asarray(inp["b1_Wq"], f32), np.asarray(inp["b2_Wq"], f32)]
    Wv = [np.asarray(inp["b1_Wv"], f32), np.asarray(inp["b2_Wv"], f32)]
    bq = [np.asarray(inp["b1_bq"], f32), np.asarray(inp["b2_bq"], f32)]
    bv = [np.asarray(inp["b1_bv"], f32), np.asarray(inp["b2_bv"], f32)]
    Wk3 = np.asarray(inp["b3_Wk"], f32)
    bk3 = np.asarray(inp["b3_bk"], f32)
    lvW = np.asarray(inp["lv_W"], f32)

    def chunk2(w):   # [256, F] -> [128, 2, F]
        return np.ascontiguousarray(w.reshape(2, 128, -1).transpose(1, 0, 2))

    def bc(v):       # [256] -> [128, 256] broadcast
        return np.broadcast_to(np.asarray(v, f32), (128, 256)).copy()

    lvw_pad = np.zeros((128, 8, 256), f16)
    for h in range(8):
        r0 = 32 * (h % 4)
        lvw_pad[r0:r0 + 32, h, :] = lvW.astype(f16)

    maps = []
    for c in range(8):
        b = c // 4
        p = c % 4
        heads = [2 * p, 2 * p + 1]
        X = [q_emb[b], qa_emb[b]]            # [2][2048, 256]

        xT = np.zeros((128, 2, 2, 2048), f16)
        wq_s = np.zeros((128, 2, 2, 2, 32), f16)
        wv_s = np.zeros((128, 2, 2, 64), f16)
        bq_col = np.zeros((32, 2, 2), f32)
        bv_col = np.zeros((32, 2, 2), f32)
        gam_col = np.zeros((128, 2, 2), f32)
        for blk in range(2):
            xT[:, :, blk, :] = X[blk].T.reshape(2, 128, 2048).transpose(
                1, 0, 2).astype(f16)
            for hi, h in enumerate(heads):
                hs = slice(32 * h, 32 * h + 32)
                wq_s[:, :, blk, hi, :] = chunk2(Wq[blk][:, hs]).astype(f16)
                bq_col[:, blk, hi] = bq[blk][hs]
                bv_col[:, blk, hi] = bv[blk][hs]
                gam_col[:, blk, hi] = gam[("b1", "b2")[blk]][h]
            wv_s[:, :, blk, :] = chunk2(
                Wv[blk][:, 64 * p:64 * p + 64]).astype(f16)

        w16 = np.zeros((128, 2, 2), f16)
        ch_col = np.zeros((1, 2), f32)
        for hi, h in enumerate(heads):
            hs = slice(32 * h, 32 * h + 32)
            w = Wk3[:, hs] @ q3[hs]
            w16[:, :, hi] = w.reshape(2, 128).T.astype(f16)
            ch_col[0, hi] = float((bk3[hs] * q3[hs]).sum() * ISQ)
        bv3_col = np.ascontiguousarray(
            np.asarray(inp["b3_bv"], f32)[64 * p:64 * p + 64].reshape(2, 32).T)

        qsel = np.zeros((128, 4), f32)
        qsel[:, p] = 1.0

        m = {
            "xT": xT,
            "wq_s": wq_s,
            "bq_col": bq_col,
            "wv_s": wv_s,
            "bv_col": bv_col,
            "gam_col": gam_col,
            "pos16": pos16,
            "spos16": spos16,
            "tri_pk": tri_pk,
            "tri_st": tri_st,
            "r1": (q_emb[b] + np.asarray(inp["b1_bo"], f32)).astype(f16),
            "r2": (qa_emb[b] + np.asarray(inp["b2_bo"], f32)).astype(f16),
            "wo1": chunk2(np.asarray(inp["b1_Wo"], f32)).astype(f16),
            "wo2": chunk2(np.asarray(inp["b2_Wo"], f32)).astype(f16),
            "g1_bc": bc(inp["b1_lng"]), "b1_bc": bc(inp["b1_lnb"]),
            "g2_bc": bc(inp["b2_lng"]), "b2_bc": bc(inp["b2_lnb"]),
            "w16": w16,
            "ch_col": ch_col,
            "wv3_s16": chunk2(np.asarray(inp["b3_Wv"], f32)
                              [:, 64 * p:64 * p + 64]).astype(f16),
            "bv3_col": bv3_col,
            "gam3_col": np.broadcast_to(gam["b3"][heads], (128, 2)).copy(),
            "wo3": chunk2(np.asarray(inp["b3_Wo"], f32)).astype(f16),
            "res3_bc": bc(know + np.asarray(inp["b3_bo"], f32)),
            "g3_bc": bc(inp["b3_lng"]), "b3_bc": bc(inp["b3_lnb"]),
            "lvw_pad16": lvw_pad,
            "lvb_bc": bc(inp["lv_b"]),
            "qrT": np.ascontiguousarray(
                q_emb[b, 512 * p:512 * p + 512].T
                .reshape(2, 128, 512).transpose(1, 0, 2)).astype(f16),
            "kpT": np.ascontiguousarray(know.reshape(8, 32).T).astype(f16),
            "lkw": np.asarray(inp["lk_W"], f32).astype(f16),
            "lkb_col": np.ascontiguousarray(
                np.asarray(inp["lk_b"], f32).reshape(2, 128).T),
            "qsel_col": qsel,
        }
        maps.append(m)
    return maps


def _ln(nc, pool, v, g_sb, b_sb, out, tag, nh1):
    """out = layernorm(v) * g + b, v fp32 [128, 256]."""
    sv = pool.tile([128, 1], dt.float32, tag=f"sv{tag}")
    nc.vector.tensor_reduce(sv[:], v[:], axis=AX.X, op=ALU.add)
    sq = pool.tile([128, 256], dt.float32, tag=f"sq{tag}")
    s2v = pool.tile([128, 1], dt.float32, tag=f"s2v{tag}")
    nc.scalar.activation(sq[:], v[:], AF.Square, bias=0.0, scale=1.0,
                         accum_out=s2v[:])
    mu = pool.tile([128, 1], dt.float32, tag=f"mu{tag}")
    nc.vector.tensor_scalar(mu[:], sv[:], 1.0 / 256, None, op0=ALU.mult)
    mu2 = pool.tile([128, 1], dt.float32, tag=f"mu2{tag}")
    nc.vector.tensor_tensor(mu2[:], mu[:], mu[:], op=ALU.mult)
    var = pool.tile([128, 1], dt.float32, tag=f"var{tag}")
    nc.vector.scalar_tensor_tensor(var[:], s2v[:], 1.0 / 256, mu2[:],
                                   op0=ALU.mult, op1=ALU.subtract)
    veps = pool.tile([128, 1], dt.float32, tag=f"veps{tag}")
    nc.vector.tensor_scalar(veps[:], var[:], 1e-5, None, op0=ALU.add)
    rstd = pool.tile([128, 1], dt.float32, tag=f"rstd{tag}")
    nc.gpsimd.tensor_tensor(rstd[:], veps[:], nh1[:], op=ALU.pow)
    xn = pool.tile([128, 256], dt.float32, tag=f"xn{tag}")
    nc.vector.tensor_scalar(xn[:], v[:], mu[:], rstd[:],
                            op0=ALU.subtract, op1=ALU.mult)
    nc.vector.tensor_tensor(xn[:], xn[:], g_sb[:], op=ALU.mult)
    nc.vector.tensor_tensor(out, xn[:], b_sb[:], op=ALU.add)


def build_bass(sim=False):
    nc = bacc.Bacc("TRN2", target_bir_lowering=False)

    def din(name, shape, dtyp=dt.float32):
        return nc.dram_tensor(name, shape, dtyp, kind="ExternalInput")

    tin = {
        "xT": din("xT", [128, 2, 2, 2048], dt.float16),
        "wq_s": din("wq_s", [128, 2, 2, 2, 32], dt.float16),
        "bq_col": din("bq_col", [32, 2, 2]),
        "wv_s": din("wv_s", [128, 2, 2, 64], dt.float16),
        "bv_col": din("bv_col", [32, 2, 2]),
        "gam_col": din("gam_col", [128, 2, 2]),
        "pos16": din("pos16", [128, 4096], dt.float16),
        "spos16": din("spos16", [128, 4096], dt.float16),
        "tri_pk": din("tri_pk", [128, 128], dt.uint8),
        "tri_st": din("tri_st", [128, 128], dt.uint8),
        "r1": din("r1", [2048, 256], dt.float16),
        "r2": din("r2", [2048, 256], dt.float16),
        "wo1": din("wo1", [128, 2, 256], dt.float16),
        "wo2": din("wo2", [128, 2, 256], dt.float16),
        "g1_bc": din("g1_bc", [128, 256]), "b1_bc": din("b1_bc", [128, 256]),
        "g2_bc": din("g2_bc", [128, 256]), "b2_bc": din("b2_bc", [128, 256]),
        "w16": din("w16", [128, 2, 2], dt.float16),
        "ch_col": din("ch_col", [1, 2]),
        "wv3_s16": din("wv3_s16", [128, 2, 64], dt.float16),
        "bv3_col": din("bv3_col", [32, 2]),
        "gam3_col": din("gam3_col", [128, 2]),
        "wo3": din("wo3", [128, 2, 256], dt.float16),
        "res3_bc": din("res3_bc", [128, 256]),
        "g3_bc": din("g3_bc", [128, 256]), "b3_bc": din("b3_bc", [128, 256]),
        "lvw_pad16": din("lvw_pad16", [128, 8, 256], dt.float16),
        "lvb_bc": din("lvb_bc", [128, 256]),
        "qrT": din("qrT", [128, 2, 512], dt.float16),
        "kpT": din("kpT", [32, 8], dt.float16),
        "lkw": din("lkw", [32, 256], dt.float16),
        "lkb_col": din("lkb_col", [128, 2]),
        "qsel_col": din("qsel_col", [128, 4]),
    }
    out_t = nc.dram_tensor("out", [512, 256], dt.float32,
                           kind="ExternalOutput")

    with tile.TileContext(nc) as tc, \
         tc.tile_pool(name="consts", bufs=1) as cs, \
         tc.tile_pool(name="dram", bufs=1, space="DRAM") as dram:
        pos = cs.tile([128, 4096], dt.float16)
        nc.sync.dma_start(pos[:], tin["pos16"][:])
        spos = cs.tile([128, 4096], dt.float16)
        nc.sync.dma_start(spos[:], tin["spos16"][:])
        tpk = cs.tile([128, 128], dt.uint8)
        nc.sync.dma_start(tpk[:], tin["tri_pk"][:])
        tst = cs.tile([128, 128], dt.uint8)
        nc.sync.dma_start(tst[:], tin["tri_st"][:])
        gamc = cs.tile([128, 2, 2], dt.float32)
        nc.sync.dma_start(gamc[:], tin["gam_col"][:])
        gam3c = cs.tile([128, 2], dt.float32)
        nc.sync.dma_start(gam3c[:], tin["gam3_col"][:])
        bqc = cs.tile([32, 2, 2], dt.float32)
        nc.sync.dma_start(bqc[:], tin["bq_col"][:])
        bvc = cs.tile([32, 2, 2], dt.float32)
        nc.sync.dma_start(bvc[:], tin["bv_col"][:])
        bv3c = cs.tile([32, 2], dt.float32)
        nc.sync.dma_start(bv3c[:], tin["bv3_col"][:])
        neg16 = cs.tile([128, 128], dt.float16)
        nc.vector.memset(neg16[:], NEG)
        zero16 = cs.tile([128, 128], dt.float16)
        nc.vector.memset(zero16[:], 0.0)
        nh1 = cs.tile([128, 1], dt.float32)
        nc.vector.memset(nh1[:], -0.5)
        onep = cs.tile([128, 1], dt.float32)
        nc.vector.memset(onep[:], 1.0 + 1e-6)

        agi = {}
        ago = {}
        for blk in range(2):
            for hi in range(2):
                agi[(blk, hi)] = dram.tile([32, 2048], dt.float16,
                                           name=f"agi{blk}{hi}")
                ago[(blk, hi)] = dram.tile([128, 2048], dt.float16,
                                           name=f"ago{blk}{hi}")
        agi3 = {hi: dram.tile([32, 2048], dt.float16, name=f"agi3{hi}")
                for hi in range(2)}
        ago3 = {hi: dram.tile([128, 2048], dt.float16, name=f"ago3{hi}")
                for hi in range(2)}
        pbuf = dram.tile([2, 2049], dt.float32)
        groups = [[0, 1, 2, 3], [4, 5, 6, 7]]

        # ---------------- projections ----------------
        QT = cs.tile([32, 2, 2, 2048], dt.float16)
        V16 = cs.tile([128, 2, 16, 64], dt.float16)
        with tc.tile_pool(name="proj", bufs=1) as pj, \
             tc.tile_pool(name="pjps", bufs=2, space="PSUM") as pjps:
            xTs = pj.tile([128, 2, 2, 2048], dt.float16)
            nc.sync.dma_start(xTs[:], tin["xT"][:])
            wqs = pj.tile([128, 2, 2, 2, 32], dt.float16)
            nc.sync.dma_start(wqs[:], tin["wq_s"][:])
            wvs = pj.tile([128, 2, 2, 64], dt.float16)
            nc.sync.dma_start(wvs[:], tin["wv_s"][:])
            for blk in range(2):
                for hi in range(2):
                    for f in range(4):
                        ps = pjps.tile([32, 512], dt.float32, tag="qt")
                        for cch in range(2):
                            nc.tensor.matmul(
                                ps[:], wqs[:, cch, blk, hi, :],
                                xTs[:, cch, blk, 512 * f:512 * f + 512],
                                start=(cch == 0), stop=(cch == 1))
                        nc.scalar.activation(
                            QT[:, blk, hi, 512 * f:512 * f + 512], ps[:],
                            AF.Identity, bias=bqc[:, blk, hi:hi + 1],
                            scale=1.0)
                for g4 in range(4):
                    vps = pjps.tile([128, 256], dt.float32, tag="v")
                    for j4 in range(4):
                        jb = 4 * g4 + j4
                        for cch in range(2):
                            nc.tensor.matmul(
                                vps[:, 64 * j4:64 * j4 + 64],
                                xTs[:, cch, blk, 128 * jb:128 * jb + 128],
                                wvs[:, cch, blk, :], start=(cch == 0),
                                stop=(cch == 1))
                    nc.vector.tensor_copy(
                        V16[:, blk, 4 * g4:4 * g4 + 4, :], vps[:])

        hq16 = cs.tile([128, 2, 2048], dt.float16)
        ha16 = cs.tile([128, 2, 2048], dt.float16)
        ctxTs = [cs.tile([128, 2, 2048], dt.float16, name=f"ctxT{w}")
                 for w in range(2)]

        # phase-2 rebuild closure (emitted interleaved with units)
        p1_cm = tc.tile_pool(name="p1", bufs=1)
        wp = p1_cm.__enter__()
        ctxps_cm = tc.tile_pool(name="ctxps", bufs=2, space="PSUM")
        ctxps = ctxps_cm.__enter__()
        scps_cm = tc.tile_pool(name="scps", bufs=2, space="PSUM")
        scps = scps_cm.__enter__()
        p2_cm = tc.tile_pool(name="p2", bufs=1)
        p2pool = p2_cm.__enter__()
        p2ps_cm = tc.tile_pool(name="p2ps", bufs=2, space="PSUM")
        p2ps = p2ps_cm.__enter__()

        def ctx_prefetch(blk):
            for cch in range(2):
                for k in range(4):
                    src = ago[(blk, k % 2)]
                    r0 = 32 * (2 * cch + k // 2)
                    nc.gpsimd.dma_start(
                        ctxTs[blk][32 * k:32 * k + 32, cch, :],
                        src[r0:r0 + 32, :])

        def p2_rebuild(which):
            wo_n = ("wo1", "wo2")[which]
            res_n = ("r1", "r2")[which]
            g_n = ("g1_bc", "g2_bc")[which]
            bb_n = ("b1_bc", "b2_bc")[which]
            dstT = (hq16, ha16)[which]
            wo_sb = p2pool.tile([128, 2, 256], dt.float16, tag=f"wo{which}")
            nc.sync.dma_start(wo_sb[:], tin[wo_n][:])
            g_sb = p2pool.tile([128, 256], dt.float32, tag=f"g{which}")
            nc.sync.dma_start(g_sb[:], tin[g_n][:])
            bb_sb = p2pool.tile([128, 256], dt.float32, tag=f"bb{which}")
            nc.sync.dma_start(bb_sb[:], tin[bb_n][:])
            ctxT = ctxTs[which]
            h16 = p2pool.tile([128, 16, 256], dt.float16, tag=f"h16{which}")
            for ic in range(16):
                tg = f"{which}{ic % 2}"
                ps = p2ps.tile([128, 256], dt.float32, tag="wops")
                for cch in range(2):
                    nc.tensor.matmul(
                        ps[:], ctxT[:, cch, 128 * ic:128 * ic + 128],
                        wo_sb[:, cch, :], start=(cch == 0), stop=(cch == 1))
                res_sb = p2pool.tile([128, 256], dt.float16, tag=f"res{tg}")
                nc.sync.dma_start(res_sb[:],
                                  tin[res_n][128 * ic:128 * ic + 128, :])
                v = p2pool.tile([128, 256], dt.float32, tag=f"v{tg}")
                nc.vector.tensor_tensor(v[:], ps[:], res_sb[:], op=ALU.add)
                _ln(nc, p2pool, v, g_sb, bb_sb, h16[:, ic, :], tg, nh1)
            for ic in range(16):
                nc.sync.dma_start_transpose(
                    dstT[:, :, 128 * ic:128 * ic + 128], h16[:, ic, :])

        def unit_b12(blk, hi):
            gam_ap = gamc[:, blk, hi:hi + 1]
            e = {}
            sc = {}
            u = {}
            invZ = wp.tile([128, 16], dt.float32, tag="invZ")
            # Two half-units (tiles 0-7, 8-15): while ACT runs one half's
            # Sqrt/Exp batches, the vector engine runs the other half's
            # scan chain -- fills both engines with no extra SBUF.
            for h0, h1 in ((0, 8), (8, NT)):
                # ---- stage A+B per tile ----
                for t in range(h0, h1):
                    W = 128 * (t + 1)
                    e[t] = wp.tile([128, W], dt.float16, tag=f"e{t}",
                                   name="e")
                    sc[t] = wp.tile([128, W], dt.float16, tag=f"sc{t}",
                                    name="sc")
                    for f0 in range(0, W, 1024):
                        fw = min(1024, W - f0)
                        ps = scps.tile([128, 1024], dt.float32, tag="sc")
                        for g in range(0, fw, 512):
                            gw = min(512, fw - g)
                            nc.tensor.matmul(
                                ps[:, g:g + gw],
                                QT[:, blk, hi, 128 * t:128 * t + 128],
                                QT[:, blk, hi, f0 + g:f0 + g + gw],
                                start=True, stop=True)
                        nc.scalar.activation(e[t][:, f0:f0 + fw], ps[:, :fw],
                                             AF.Exp, bias=0.0, scale=ISQ)
                        nc.vector.tensor_scalar(sc[t][:, f0:f0 + fw],
                                                ps[:, :fw], ISQ, None,
                                                op0=ALU.mult)
                    nc.vector.copy_predicated(e[t][:, W - 128:W], tpk[:],
                                              zero16[:])
                    suf = wp.tile([128, 2049], dt.float16, tag=f"suf{t % 3}",
                                  name="suf")
                    nc.vector.memset(suf[:, W:W + 1], 0.0)
                    nc.vector.tensor_tensor_scan(
                        suf[:, :W][:, ::-1], e[t][:, :W][:, ::-1],
                        e[t][:, :W][:, ::-1], 0.0, op0=ALU.add,
                        op1=ALU.bypass)
                    nc.vector.reciprocal(invZ[:, t:t + 1], suf[:, 0:1])
                    u[t] = wp.tile([128, W], dt.float16, tag=f"e{t}",
                                   name="u")
                    nc.vector.tensor_tensor(
                        u[t][:], suf[:, 1:W + 1],
                        pos[:, 2048 - 128 * t:2048 - 128 * t + W],
                        op=ALU.mult)
                # ---- stage C: sqrt batch (in-place RMW on u) ----
                # gate: zeros column depending on the half's LAST stage-B
                # reciprocal (always finite; u itself may hold fp16 inf), so
                # all Sqrts turn ready together and the greedy scheduler
                # cannot interleave Exps into the Sqrt table batch
                gate = wp.tile([128, 1], dt.float32, tag="gate")
                nc.vector.tensor_scalar(gate[:], invZ[:, h1 - 1:h1], 0.0,
                                        None, op0=ALU.mult)
                for t in range(h0, h1):
                    nc.scalar.activation(u[t][:], u[t][:], AF.Sqrt,
                                         bias=gate[:],
                                         scale=invZ[:, t:t + 1])
                # ---- stage D: m0 exp batch (in-place RMW on u) ----
                for t in range(h0, h1):
                    nc.scalar.activation(u[t][:], u[t][:], AF.Exp, bias=0.0,
                                         scale=gam_ap)
                # ---- stage E: s2 (in-place), e2+Z (ACT), p2, ctx ----
                for t in range(h0, h1):
                    W = 128 * (t + 1)
                    nc.vector.tensor_tensor(sc[t][:], u[t][:], sc[t][:],
                                            op=ALU.mult)
                    nc.vector.copy_predicated(sc[t][:, W - 128:W], tpk[:],
                                              neg16[:])
                    e2 = wp.tile([128, W], dt.float16, tag=f"e{t}",
                                 name="e2")
                    Z2 = wp.tile([128, 1], dt.float32, tag="Z2")
                    nc.scalar.activation(e2[:], sc[t][:], AF.Exp, bias=0.0,
                                         scale=1.0, accum_out=Z2[:])
                    iZ2 = wp.tile([128, 1], dt.float32, tag="iZ2")
                    nc.vector.reciprocal(iZ2[:], Z2[:])
                    p2t = wp.tile([128, W], dt.float16, tag=f"sc{t}",
                                  name="p2")
                    nc.vector.tensor_scalar(p2t[:], e2[:], iZ2[:], None,
                                            op0=ALU.mult)
                    p2T = wp.tile([128, 16, 128], dt.float16,
                                  tag=f"eT{t % 3}", name="p2T")
                    nc.sync.dma_start_transpose(p2T[:, :t + 1, :],
                                                p2t[:, :W])
                    cps = ctxps.tile([32, 128], dt.float32, tag="ctx")
                    for jb in range(t + 1):
                        nc.tensor.matmul(
                            cps[:], V16[:, blk, jb, 32 * hi:32 * hi + 32],
                            p2T[:, jb, :], start=(jb == 0), stop=(jb == t))
                    csb = wp.tile([32, 128], dt.float16, tag="csb",
                                  name="csb")
                    nc.vector.tensor_scalar(csb[:], cps[:],
                                            bvc[:, blk, hi:hi + 1], None,
                                            op0=ALU.add)
                    nc.sync.dma_start(
                        agi[(blk, hi)][0:32, 128 * t:128 * t + 128], csb[:])
            agx, agy = agi[(blk, hi)], ago[(blk, hi)]
            if sim:
                for rr in range(4):
                    nc.gpsimd.dma_start(agy[32 * rr:32 * rr + 32, :], agx[:])
            else:
                nc.gpsimd.collective_compute(
                    "AllGather", ALU.bypass, ins=[agx.opt()],
                    outs=[agy.opt()], replica_groups=groups)

        unit_b12(0, 0)
        unit_b12(0, 1)
        unit_b12(1, 0)
        ctx_prefetch(0)
        p2_rebuild(0)          # hq — its vector work overlaps unit (1,1)
        unit_b12(1, 1)
        ctx_prefetch(1)
        p2_rebuild(1)          # ha — executes while unit (1,1) tail drains

        p2ps_cm.__exit__(None, None, None)
        p2_cm.__exit__(None, None, None)
        scps_cm.__exit__(None, None, None)

        # ---------------- block3 ----------------
        b3_cm = tc.tile_pool(name="b3", bufs=1)
        b3p = b3_cm.__enter__()
        b3ps_cm = tc.tile_pool(name="b3ps", bufs=2, space="PSUM")
        ps3 = b3ps_cm.__enter__()

        w16sb = b3p.tile([128, 2, 2], dt.float16, tag="w16")
        nc.sync.dma_start(w16sb[:], tin["w16"][:])
        chc = b3p.tile([1, 2], dt.float32, tag="chc")
        nc.sync.dma_start(chc[:], tin["ch_col"][:])
        wv3 = b3p.tile([128, 2, 64], dt.float16, tag="wv3")
        nc.sync.dma_start(wv3[:], tin["wv3_s16"][:])
        one1 = b3p.tile([1, 1], dt.float32, tag="one1")
        nc.vector.memset(one1[:], 1.0)
        onesrow = b3p.tile([1, 128], dt.float32, tag="onesrow")
        nc.vector.memset(onesrow[:], 1.0)
        V3 = b3p.tile([128, 16, 64], dt.float16, tag="V3")

        def unit_b3(hi):
            gam_ap = gam3c[:, hi:hi + 1]
            s_row = b3p.tile([1, 2048], dt.float32, tag="srow")
            for f in range(4):
                sp = ps3.tile([1, 512], dt.float32, tag="s")
                for cch in range(2):
                    nc.tensor.matmul(
                        sp[:], w16sb[:, cch, hi:hi + 1],
                        hq16[:, cch, 512 * f:512 * f + 512],
                        start=(cch == 0), stop=(cch == 1))
                nc.scalar.activation(
                    s_row[:, 512 * f:512 * f + 512], sp[:], AF.Identity,
                    bias=chc[:, hi:hi + 1], scale=ISQ)
            smax = b3p.tile([1, 1], dt.float32, tag="smax")
            nc.vector.tensor_reduce(smax[:], s_row[:], axis=AX.X, op=ALU.max)
            nsmax = b3p.tile([1, 1], dt.float32, tag="nsmax")
            nc.vector.tensor_scalar(nsmax[:], smax[:], -1.0, None,
                                    op0=ALU.mult)
            e3 = b3p.tile([1, 2048], dt.float32, tag="e3")
            nc.scalar.activation(e3[:], s_row[:], AF.Exp, bias=nsmax[:],
                                 scale=1.0)
            P_row = b3p.tile([1, 2048], dt.float32, tag="Prow")
            nc.vector.tensor_tensor_scan(P_row[:], e3[:], e3[:], 0.0,
                                         op0=ALU.add, op1=ALU.bypass)
            nc.sync.dma_start(pbuf[hi, 0:1], one1[:])
            nc.sync.dma_start(pbuf[hi, 1:2049], P_row[:])
            npcol = b3p.tile([128, 16], dt.float32, tag="npcol")
            pcol = b3p.tile([128, 16], dt.float32, tag="pcol")
            nc.sync.dma_start(
                pcol[:], pbuf[hi, 0:2048].rearrange("(t p) -> p t", p=128))
            nc.vector.tensor_scalar(pcol[:], pcol[:], -1.0, None,
                                    op0=ALU.mult)
            nc.vector.reciprocal(npcol[:], pcol[:])
            P_bc = b3p.tile([128, 2048], dt.float32, tag="Pbc")
            s_bc = b3p.tile([128, 2048], dt.float16, tag="sbc")
            for f in range(4):
                bp = ps3.tile([128, 512], dt.float32, tag="bc")
                nc.tensor.matmul(bp[:], onesrow[:],
                                 P_row[:, 512 * f:512 * f + 512],
                                 start=True, stop=True)
                nc.vector.tensor_copy(P_bc[:, 512 * f:512 * f + 512], bp[:])
                bs = ps3.tile([128, 512], dt.float32, tag="bc")
                nc.tensor.matmul(bs[:], onesrow[:],
                                 s_row[:, 512 * f:512 * f + 512],
                                 start=True, stop=True)
                nc.vector.tensor_copy(s_bc[:, 512 * f:512 * f + 512], bs[:])
            if hi == 0:
                # V3 projection (shared by both heads; needs ha16)
                for g4 in range(4):
                    vps = ps3.tile([128, 256], dt.float32, tag="v3")
                    for j4 in range(4):
                        jb = 4 * g4 + j4
                        for cch in range(2):
                            nc.tensor.matmul(
                                vps[:, 64 * j4:64 * j4 + 64],
                                ha16[:, cch, 128 * jb:128 * jb + 128],
                                wv3[:, cch, :], start=(cch == 0),
                                stop=(cch == 1))
                    nc.vector.tensor_copy(V3[:, 4 * g4:4 * g4 + 4, :],
                                          vps[:])
            # ---- sqrt batch: rt = sqrt(1+eps - P_j/P_i) ----
            gate3 = wp.tile([128, 1], dt.float32, tag="gate")
            nc.vector.scalar_tensor_tensor(gate3[:], s_bc[:, 0:1], 0.0,
                                           onep[:], op0=ALU.mult, op1=ALU.add)
            rt = {}
            for t in range(NT):
                W = 128 * (t + 1)
                rt[t] = wp.tile([128, W], dt.float16, tag=f"e{t}", name="rt")
                nc.scalar.activation(rt[t][:], P_bc[:, :W], AF.Sqrt,
                                     bias=gate3[:], scale=npcol[:, t:t + 1])
            # ---- r = rt * spos (vector TT 2x) ----
            rr = {}
            for t in range(NT):
                W = 128 * (t + 1)
                rr[t] = wp.tile([128, W], dt.float16, tag=f"sc{t}",
                                name="r3")
                nc.vector.tensor_tensor(
                    rr[t][:], rt[t][:],
                    spos[:, 2048 - 128 * t:2048 - 128 * t + W], op=ALU.mult)
            # ---- m0 exp batch ----
            m0 = {}
            for t in range(NT):
                m0[t] = wp.tile([128, 128 * (t + 1)], dt.float16,
                                tag=f"e{t}", name="m0b3")
                nc.scalar.activation(m0[t][:], rr[t][:], AF.Exp, bias=0.0,
                                     scale=gam_ap)
            # ---- stage E ----
            for t in range(NT):
                W = 128 * (t + 1)
                s2 = wp.tile([128, W], dt.float16, tag=f"sc{t}", name="s2b3")
                nc.vector.tensor_tensor(s2[:], m0[t][:], s_bc[:, :W],
                                        op=ALU.mult)
                nc.vector.copy_predicated(s2[:, W - 128:W], tst[:],
                                          neg16[:])
                e2 = wp.tile([128, W], dt.float16, tag=f"e{t}", name="e2b3")
                Z2 = wp.tile([128, 1], dt.float32, tag="Z2")
                nc.scalar.activation(e2[:], s2[:], AF.Exp, bias=0.0,
                                     scale=1.0, accum_out=Z2[:])
                iZ2 = wp.tile([128, 1], dt.float32, tag="iZ2")
                nc.vector.reciprocal(iZ2[:], Z2[:])
                p2t = wp.tile([128, W], dt.float16, tag=f"sc{t}",
                              name="p2b3")
                nc.vector.tensor_scalar(p2t[:], e2[:], iZ2[:], None,
                                        op0=ALU.mult)
                if t == 0:
                    nc.vector.memset(p2t[0:1, 0:128], 0.0)
                p2T = wp.tile([128, 16, 128], dt.float16, tag=f"eT{t % 3}",
                              name="p2Tb3")
                nc.sync.dma_start_transpose(p2T[:, :t + 1, :], p2t[:, :W])
                cps = ctxps.tile([32, 128], dt.float32, tag="ctx")
                for jb in range(t + 1):
                    nc.tensor.matmul(
                        cps[:], V3[:, jb, 32 * hi:32 * hi + 32],
                        p2T[:, jb, :], start=(jb == 0), stop=(jb == t))
                csb = wp.tile([32, 128], dt.float16, tag="csb", name="csb3")
                nc.vector.tensor_scalar(csb[:], cps[:], bv3c[:, hi:hi + 1],
                                        None, op0=ALU.add)
                nc.sync.dma_start(agi3[hi][0:32, 128 * t:128 * t + 128],
                                  csb[:])
            agx, agy = agi3[hi], ago3[hi]
            if sim:
                for rw in range(4):
                    nc.gpsimd.dma_start(agy[32 * rw:32 * rw + 32, :], agx[:])
            else:
                nc.gpsimd.collective_compute(
                    "AllGather", ALU.bypass, ins=[agx.opt()],
                    outs=[agy.opt()], replica_groups=groups)

        unit_b3(0)
        unit_b3(1)

        b3ps_cm.__exit__(None, None, None)
        b3_cm.__exit__(None, None, None)
        scps2 = None
        ctxps_cm.__exit__(None, None, None)
        p1_cm.__exit__(None, None, None)

        # ---------------- phase 3 ----------------
        with tc.tile_pool(name="p3", bufs=1) as wp3, \
             tc.tile_pool(name="p3ps", bufs=2, space="PSUM") as ps:
            wo3 = wp3.tile([128, 2, 256], dt.float16, tag="wo3")
            nc.sync.dma_start(wo3[:], tin["wo3"][:])
            res3 = wp3.tile([128, 256], dt.float32, tag="res3")
            nc.sync.dma_start(res3[:], tin["res3_bc"][:])
            g3 = wp3.tile([128, 256], dt.float32, tag="g3")
            nc.sync.dma_start(g3[:], tin["g3_bc"][:])
            b3 = wp3.tile([128, 256], dt.float32, tag="b3")
            nc.sync.dma_start(b3[:], tin["b3_bc"][:])
            lvw = wp3.tile([128, 8, 256], dt.float16, tag="lvw")
            nc.sync.dma_start(lvw[:], tin["lvw_pad16"][:])
            lvb = wp3.tile([128, 256], dt.float32, tag="lvb")
            nc.sync.dma_start(lvb[:], tin["lvb_bc"][:])
            qrTs = wp3.tile([128, 2, 512], dt.float16, tag="qrTs")
            nc.sync.dma_start(qrTs[:], tin["qrT"][:])
            kpTs = wp3.tile([32, 8], dt.float16, tag="kpTs")
            nc.sync.dma_start(kpTs[:], tin["kpT"][:])
            lkws = wp3.tile([32, 256], dt.float16, tag="lkws")
            nc.sync.dma_start(lkws[:], tin["lkw"][:])
            lkbc = wp3.tile([128, 2], dt.float32, tag="lkbc")
            nc.sync.dma_start(lkbc[:], tin["lkb_col"][:])
            qsel = wp3.tile([128, 4], dt.float32, tag="qsel")
            nc.sync.dma_start(qsel[:], tin["qsel_col"][:])

            keyT = wp3.tile([128, 2, 8], dt.float16, tag="keyT")
            for cch in range(2):
                kps = ps.tile([128, 8], dt.float32, tag="key")
                nc.tensor.matmul(kps[:], lkws[:, 128 * cch:128 * cch + 128],
                                 kpTs[:], start=True, stop=True)
                nc.scalar.activation(keyT[:, cch, :], kps[:], AF.Sigmoid,
                                     bias=lkbc[:, cch:cch + 1], scale=1.0)

            alphas = []
            for ic in range(4):
                bps = ps.tile([128, 8], dt.float32, tag="beta")
                for cch in range(2):
                    nc.tensor.matmul(
                        bps[:], qrTs[:, cch, 128 * ic:128 * ic + 128],
                        keyT[:, cch, :], start=(cch == 0), stop=(cch == 1))
                bmax = wp3.tile([128, 1], dt.float32, tag=f"bmax{ic % 2}")
                nc.vector.tensor_reduce(bmax[:], bps[:], axis=AX.X,
                                        op=ALU.max)
                nbmax = wp3.tile([128, 1], dt.float32, tag=f"nbmax{ic % 2}")
                nc.vector.tensor_scalar(nbmax[:], bmax[:], -1.0, None,
                                        op0=ALU.mult)
                ebeta = wp3.tile([128, 8], dt.float32, tag=f"ebeta{ic % 2}")
                zb = wp3.tile([128, 1], dt.float32, tag=f"zb{ic % 2}")
                nc.scalar.activation(ebeta[:], bps[:], AF.Exp, bias=nbmax[:],
                                     scale=1.0, accum_out=zb[:])
                izb = wp3.tile([128, 1], dt.float32, tag=f"izb{ic % 2}")
                nc.vector.reciprocal(izb[:], zb[:])
                alpha = wp3.tile([128, 8], dt.float32, tag=f"alpha{ic}")
                nc.vector.tensor_scalar(alpha[:], ebeta[:], izb[:], None,
                                        op0=ALU.mult)
                alphas.append(alpha)

            ag2f = wp3.tile([128, 2, 2048], dt.float16, tag="ag2f")
            for cch in range(2):
                for k in range(4):
                    r0 = 32 * (2 * cch + k // 2)
                    nc.sync.dma_start(ag2f[32 * k:32 * k + 32, cch, :],
                                      ago3[k % 2][r0:r0 + 32, :])
            ag2sb = wp3.tile([128, 2, 512], dt.float16, tag="ag2sb")
            for cch in range(2):
                blendt = wp3.tile([128, 512], dt.float16, tag="blendt")
                nc.vector.tensor_scalar(
                    blendt[:], ag2f[:, cch, 0:512], qsel[:, 0:1], None,
                    op0=ALU.mult)
                for qq in range(1, 4):
                    dst2 = blendt[:] if qq < 3 else ag2sb[:, cch, :]
                    nc.vector.scalar_tensor_tensor(
                        dst2, ag2f[:, cch, 512 * qq:512 * qq + 512],
                        qsel[:, qq:qq + 1], blendt[:],
                        op0=ALU.mult, op1=ALU.add)

            for ic in range(4):
                tg = f"{ic % 2}"
                wops = ps.tile([128, 256], dt.float32, tag="wo3ps")
                for cch in range(2):
                    nc.tensor.matmul(
                        wops[:], ag2sb[:, cch, 128 * ic:128 * ic + 128],
                        wo3[:, cch, :], start=(cch == 0), stop=(cch == 1))
                v = wp3.tile([128, 256], dt.float32, tag=f"v3p{tg}")
                nc.vector.tensor_tensor(v[:], wops[:], res3[:], op=ALU.add)
                h3 = wp3.tile([128, 256], dt.float32, tag=f"h3{tg}")
                _ln(nc, wp3, v, g3, b3, h3[:], "3" + tg, nh1)
                h316 = wp3.tile([128, 256], dt.float16, tag=f"h316{tg}")
                nc.vector.tensor_copy(h316[:], h3[:])
                h3T = wp3.tile([128, 2, 128], dt.float16, tag=f"h3T{tg}")
                nc.sync.dma_start_transpose(h3T[:], h316[:])

                alpha = alphas[ic]
                acc = wp3.tile([128, 256], dt.float32, tag=f"acc{tg}")
                accb = wp3.tile([128, 256], dt.float32, tag=f"accb{tg}")
                nc.vector.memset(acc[:], 0.0)
                for h in range(8):
                    vps = ps.tile([128, 256], dt.float32, tag="valps")
                    nc.tensor.matmul(vps[:], h3T[:, h // 4, :], lvw[:, h, :],
                                     start=True, stop=True)
                    val = wp3.tile([128, 256], dt.float32, tag=f"val{tg}")
                    nc.vector.tensor_tensor(val[:], vps[:], lvb[:],
                                            op=ALU.add)
                    vsg = wp3.tile([128, 256], dt.float32, tag=f"vsg{tg}")
                    nc.scalar.activation(vsg[:], val[:], AF.Sigmoid,
                                         bias=0.0, scale=1.0)
                    src, dst2 = (acc, accb) if h % 2 == 0 else (accb, acc)
                    nc.vector.scalar_tensor_tensor(
                        dst2[:], vsg[:], alpha[:, h:h + 1], src[:],
                        op0=ALU.mult, op1=ALU.add)
                nc.sync.dma_start(out_t[128 * ic:128 * ic + 128, :], acc[:])

    nc.finalize()
    return nc


def run(inputs, **kw):
    if "nc" not in _BUILT:
        _BUILT["nc"] = build_bass()
    nc = _BUILT["nc"]
    in_maps = build_in_maps(inputs)
    res = bass_utils.run_bass_kernel_spmd(nc, in_maps,
                                          core_ids=list(range(8)), **kw)
    out = np.zeros((2, 2048, 256), np.float32)
    for c in range(8):
        b, q = c // 4, c % 4
        out[b, 512 * q:512 * q + 512, :] = res.results[c]["out"]
    return out, res


def kernel(**inputs):
    return run(inputs)[0]
